# revision 12
# baseline (speedup 1.0000x reference)
"""Trainium2 Bass kernel for one neural-CA (NCA) update step.

Model (per batch element, all f32):
  pre_life  = living_mask(x)                        # 3x3 circular max/avg pools on alpha=x[:,3]
  y         = depthwise 3x3 circular conv of x with 4 filters  -> [C*4, H, W]
  h         = leaky_relu(W1 @ y + b1, 0.01)         # per-pixel MLP, HID=128
  dx        = W2 @ h + b2
  xnew      = x + dx * (rand_mask <= 0.5)
  post_life = living_mask(xnew)
  out       = xnew * (pre_life & post_life)

Strategy (8 NeuronCores, pure data parallel over batch 32 -> 4 per core):
  * Fold conv+W1 into effective weights Weff[o, c, di, dj] (host precompute).
  * Keep a 3-replica row-shifted padded stack xr[48 = 3*16ch, 66*130] in SBUF
    (half-batch granularity); the 3x3 conv+MLP1 becomes 3 accumulating K=48
    matmuls per 512-pixel chunk whose rhs are column-shifted views of xr.
  * Lrelu+bias on ScalarE straight out of PSUM.
  * MLP2 (K=128, M=16->32 zero-padded) col-tiled via tile_position: 4 chunks
    run concurrently in one PSUM bank; one VectorE op evacuates + adds b2.
  * dx bounces through a DRAM scratch into an H-major layout [H, C*W]; the
    elementwise tail + life-mask pools run there with 128-partition tiles and
    free-dim-broadcast per-pixel masks.
  * Matmul dtype switchable f32 <-> f32r (bitcast) for PE speed.
"""

import os
import sys

os.environ.setdefault("JAX_PLATFORMS", "cpu")
for _p in ("/opt/trn_rl_repo", "/root/.axon_site/_ro/trn_rl_repo"):
    if os.path.isdir(_p) and _p not in sys.path:
        sys.path.insert(0, _p)

from contextlib import ExitStack

import numpy as np

import concourse.bass as bass
import concourse.tile as tile
from concourse import bacc, mybir
from concourse._compat import with_exitstack
from concourse.bass_utils import run_bass_kernel_spmd

# ----------------------------------------------------------------------------
# problem constants (hardcoded per spec nn_CAModel_2121713844629)
B, C, H, W = 32, 16, 128, 128
NF, R, K = 4, 1, 3
HID = 128
FIRE_RATE = 0.5
NEG_SLOPE = 0.01
N_CORES = 8
B_LOC = B // N_CORES          # 4 batches per core
ROWS_PER_CHUNK = 4            # 4 image rows = 512 pixels per matmul chunk
CHUNK = ROWS_PER_CHUNK * W    # 512
N_CHUNKS = H // ROWS_PER_CHUNK                 # 32 per batch
GROUPS = N_CHUNKS // 4        # 8 col-tiled MLP2 groups per batch
HALF_ROWS = 64                # image rows per stack half
SROWS = HALF_ROWS + 2         # 66 stack rows (1 halo row each side)
SW = W + 2                    # 130 padded row width
ST = SROWS * SW               # stack free size per partition
CW = C * W                    # 2048, EW free size
PADROWS = H + 4               # host-padded image rows (x rows -2..129)
PADT = PADROWS * SW           # host-padded flat size per channel

MM_DTYPE = os.environ.get("CA_MM_DTYPE", "bf16hl")  # "bf16hl", "f32r" or "f32"
LRELU_MODE = os.environ.get("CA_LRELU", "act")    # "act" (HW Lrelu) or "decomp" (sim-safe)

F32 = mybir.dt.float32
BF16 = mybir.dt.bfloat16
F32MM = mybir.dt.float32r if MM_DTYPE == "f32r" else mybir.dt.float32


def _mm(ap):
    """View an AP in the matmul input dtype."""
    if MM_DTYPE == "f32r":
        return ap.bitcast(mybir.dt.float32r)
    return ap


def _avg_threshold():
    """Smallest f32 s with (np.float32(s)/9 < 0.2) False, as the strict-< bound.

    reference computes (sum/9 < 0.2); we compare (sum < s*) with s* chosen so
    the predicates agree for every f32 sum value.
    """
    lo = np.float32(1.7)
    hi = np.float32(1.9)
    # binary search over f32 values
    for _ in range(80):
        mid = np.float32((lo.astype(np.float64) + hi.astype(np.float64)) / 2)
        if mid / np.float32(9.0) < np.float32(0.2):
            lo = mid
        else:
            hi = mid
    # predicate (sum/9 < 0.2)  <=>  (sum < hi)
    return float(hi)


AVG_LT = _avg_threshold()


# ----------------------------------------------------------------------------
@with_exitstack
def _build_kernel(ctx: ExitStack, tc: "tile.TileContext",
                  xpad_in, xpadl_in, xew_in, m_in, wa_in, w2_in, b1_in, b2_in,
                  nb1_in, out_dram, scr_drams):
    nc = tc.nc
    consts = ctx.enter_context(tc.tile_pool(name="consts", bufs=1))
    stacks = ctx.enter_context(tc.tile_pool(name="stacks", bufs=2))
    hpool = ctx.enter_context(tc.tile_pool(name="hpool", bufs=6))
    ewpool = ctx.enter_context(tc.tile_pool(name="ewpool", bufs=2))
    small = ctx.enter_context(tc.tile_pool(name="small", bufs=1))
    psum_h = ctx.enter_context(tc.tile_pool(name="psum_h", bufs=3, space="PSUM"))
    psum_dx = ctx.enter_context(tc.tile_pool(name="psum_dx", bufs=1, space="PSUM"))

    # --- constants ----------------------------------------------------------
    if MM_DTYPE == "bf16hl":
        # rows 0-47: bf16(WA) ("hi"); rows 48-95: bf16(WA - hi) ("lo")
        wa_t = consts.tile([6 * C, 3 * HID], BF16)
    else:
        wa_t = consts.tile([3 * C, 3 * HID], F32MM)
    w2_t = consts.tile([HID, 32], F32MM)            # W2^T zero-padded to M=32
    b1_t = consts.tile([HID, 1], F32)
    b2_t = consts.tile([HID, 1], F32)               # b2 replicated at 32j+c
    nc.sync.dma_start(wa_t[:], _mm(wa_in[:]))
    nc.sync.dma_start(w2_t[:], _mm(w2_in[:]))
    nc.sync.dma_start(b1_t[:], b1_in[:])
    nc.sync.dma_start(b2_t[:], b2_in[:])
    if LRELU_MODE == "decomp":
        nb1_t = consts.tile([HID, 1], F32)
        nc.sync.dma_start(nb1_t[:], nb1_in[:])
    m_all = consts.tile([H, B_LOC * W], F32)
    nc.sync.dma_start(m_all[:], m_in[:])

    for b in range(B_LOC):
        scr = scr_drams[b]

        # --- EW-layout input [H, C*W] (host-pretransposed) ------------------
        x_ew = ewpool.tile([H, CW], F32, name=f"x_ew{b}", tag="x_ew")
        nc.sync.dma_start(x_ew[:], xew_in[b])

        # --- matmul pipeline over 2 stack halves ---------------------------
        h_tiles = {}
        for s in range(2):
            # stack row r_loc of replica d = x row (64s + r_loc + d - 2)
            #  -> xpad row (64s + r_loc + d), xpad rows = x rows -2..129
            if MM_DTYPE == "bf16hl":
                # hi-stack twice (partitions 0-47 / 48-95) + lo-stack (0-47)
                xr = stacks.tile([6 * C, ST], BF16, name=f"xr{b}_{s}", tag="xr")
                xrl = stacks.tile([3 * C, ST], BF16, name=f"xrl{b}_{s}", tag="xrl")
                for d in range(3):
                    win = HALF_ROWS * s + d
                    srcap = bass.AP(
                        tensor=xpad_in.tensor,
                        offset=xpad_in.offset + (b * C) * PADT + win * SW,
                        ap=[[PADT, C], [1, ST]])
                    for rep in range(2):
                        dstap = bass.AP(tensor=xr.tensor,
                                        offset=xr.offset + (rep * 3 + d) * C * ST,
                                        ap=[[ST, C], [1, ST]])
                        nc.sync.dma_start(dstap, srcap)
                    srcl = bass.AP(
                        tensor=xpadl_in.tensor,
                        offset=xpadl_in.offset + (b * C) * PADT + win * SW,
                        ap=[[PADT, C], [1, ST]])
                    dstl = bass.AP(tensor=xrl.tensor, offset=xrl.offset + d * C * ST,
                                   ap=[[ST, C], [1, ST]])
                    nc.sync.dma_start(dstl, srcl)
            else:
                xr = stacks.tile([3 * C, ST], F32MM, name=f"xr{b}_{s}", tag="xr")
                for d in range(3):
                    win = HALF_ROWS * s + d
                    srcap = bass.AP(
                        tensor=xpad_in.tensor,
                        offset=xpad_in.offset + (b * C) * PADT + win * SW,
                        ap=[[PADT, C], [1, ST]])
                    dstap = bass.AP(tensor=xr.tensor, offset=xr.offset + d * C * ST,
                                    ap=[[ST, C], [1, ST]])
                    nc.sync.dma_start(dstap, _mm(srcap))

            for cl in range(N_CHUNKS // 2):          # 16 chunks per half
                cg = s * (N_CHUNKS // 2) + cl
                h_ps = psum_h.tile([HID, CHUNK], F32, name=f"hps{b}_{cg}", tag="h_ps")
                base = (ROWS_PER_CHUNK * cl + 1) * SW + 1
                if MM_DTYPE == "bf16hl":
                    for idx in range(3):             # dj = idx - 1
                        # hi*HI + lo*HI  (K = 96: [Wh; Wl] @ [xh; xh])
                        rhs = bass.AP(tensor=xr.tensor,
                                      offset=xr.offset + base + (idx - 1),
                                      ap=[[ST, 6 * C], [SW, ROWS_PER_CHUNK], [1, W]])
                        nc.tensor.matmul(h_ps[:],
                                         wa_t[:, idx * HID:(idx + 1) * HID],
                                         rhs,
                                         start=(idx == 0), stop=False)
                        # hi*LO  (K = 48: Wh @ xl)
                        rhsl = bass.AP(tensor=xrl.tensor,
                                       offset=xrl.offset + base + (idx - 1),
                                       ap=[[ST, 3 * C], [SW, ROWS_PER_CHUNK], [1, W]])
                        nc.tensor.matmul(h_ps[:],
                                         wa_t[0:3 * C, idx * HID:(idx + 1) * HID],
                                         rhsl,
                                         start=False, stop=(idx == 2))
                else:
                    for idx in range(3):             # dj = idx - 1
                        rhs = bass.AP(tensor=xr.tensor,
                                      offset=xr.offset + base + (idx - 1),
                                      ap=[[ST, 3 * C], [SW, ROWS_PER_CHUNK], [1, W]])
                        nc.tensor.matmul(h_ps[:],
                                         wa_t[:, idx * HID:(idx + 1) * HID],
                                         rhs,
                                         start=(idx == 0), stop=(idx == 2))
                h_sb = hpool.tile([HID, CHUNK], F32MM, name=f"h{b}_{cg}", tag="h_sb")
                if LRELU_MODE == "act":
                    nc.scalar.activation(h_sb[:], h_ps[:],
                                         mybir.ActivationFunctionType.Lrelu,
                                         bias=b1_t[:], scale=1.0, alpha=NEG_SLOPE)
                else:
                    # lrelu(v) = relu(v) - slope * relu(-v), v = h + b1
                    rpos = hpool.tile([HID, CHUNK], F32, name=f"rp{b}_{cg}", tag="rpos", bufs=2)
                    rneg = hpool.tile([HID, CHUNK], F32, name=f"rn{b}_{cg}", tag="rneg", bufs=2)
                    nc.scalar.activation(rpos[:], h_ps[:],
                                         mybir.ActivationFunctionType.Relu,
                                         bias=b1_t[:], scale=1.0)
                    nc.scalar.activation(rneg[:], h_ps[:],
                                         mybir.ActivationFunctionType.Relu,
                                         bias=nb1_t[:], scale=-1.0)
                    nc.vector.tensor_scalar(rneg[:], rneg[:], -NEG_SLOPE, None,
                                            op0=mybir.AluOpType.mult)
                    nc.vector.tensor_tensor(h_sb[:], rpos[:], rneg[:],
                                            op=mybir.AluOpType.add)
                h_tiles[cg] = h_sb

                if cg % 4 == 3:                      # one MLP2 group ready
                    t = cg // 4
                    dx_ps = psum_dx.tile([C, 4 * CHUNK], F32, name=f"dxps{b}_{t}", tag="dx_ps")
                    for j in range(4):
                        nc.tensor.matmul(dx_ps[:, j * CHUNK:(j + 1) * CHUNK],
                                         w2_t[:, 0:C], h_tiles[t * 4 + j][:],
                                         start=True, stop=True)
                    dxs = hpool.tile([C, 4 * CHUNK], F32, name=f"dxs{b}_{t}", tag="dxs", bufs=2)
                    if t % 2 == 0:
                        nc.vector.tensor_scalar(dxs[:], dx_ps[:], b2_t[0:C, :], None,
                                                op0=mybir.AluOpType.add)
                    else:
                        nc.scalar.activation(dxs[:], dx_ps[:],
                                             mybir.ActivationFunctionType.Identity,
                                             bias=b2_t[0:C, :], scale=1.0)
                    # dump into H-major DRAM scratch [H, C, W]
                    for j in range(4):
                        srcap = bass.AP(tensor=dxs.tensor,
                                        offset=dxs.offset + j * CHUNK,
                                        ap=[[4 * CHUNK, C], [W, ROWS_PER_CHUNK], [1, W]])
                        dstap = bass.AP(tensor=scr.tensor,
                                        offset=scr.offset + (16 * t + 4 * j) * CW,
                                        ap=[[W, C], [CW, ROWS_PER_CHUNK], [1, W]])
                        eng = nc.sync if j % 2 == 0 else nc.scalar
                        eng.dma_start(dstap, srcap)

        # --- elementwise tail in [H, C*W] ----------------------------------
        dx_ew = ewpool.tile([H, CW], F32, name=f"dx_ew{b}", tag="dx_ew")
        nc.scalar.dma_start(dx_ew[:], scr.rearrange("h c w -> h (c w)"))

        def bcast(t128):
            return bass.AP(tensor=t128.tensor, offset=t128.offset,
                           ap=[[t128.ap[0][0], H], [0, C], [1, W]])

        m_b = bass.AP(tensor=m_all.tensor, offset=m_all.offset + b * W,
                      ap=[[m_all.ap[0][0], H], [0, C], [1, W]])
        nc.vector.tensor_tensor(dx_ew[:], dx_ew[:], m_b, op=mybir.AluOpType.mult)
        xnew = ewpool.tile([H, CW], F32, name=f"xnew{b}", tag="xnew")
        nc.vector.tensor_tensor(xnew[:], x_ew[:], dx_ew[:], op=mybir.AluOpType.add)

        # --- life masks ----------------------------------------------------
        def living(src_ew, which):
            ap_pad = small.tile([H, SW], F32, name=f"ap{which}{b}", tag=f"ap{which}")
            alpha = src_ew[:, 3 * W:4 * W]
            nc.vector.tensor_copy(ap_pad[:, 1:1 + W], alpha)
            nc.vector.tensor_copy(ap_pad[:, 0:1], src_ew[:, 4 * W - 1:4 * W])
            nc.vector.tensor_copy(ap_pad[:, 1 + W:2 + W], src_ew[:, 3 * W:3 * W + 1])
            hm = small.tile([H, W], F32, name=f"hm{which}{b}", tag=f"hm{which}")
            hs = small.tile([H, W], F32, name=f"hs{which}{b}", tag=f"hs{which}")
            nc.vector.tensor_tensor(hm[:], ap_pad[:, 0:W], ap_pad[:, 1:1 + W],
                                    op=mybir.AluOpType.max)
            nc.vector.tensor_tensor(hm[:], hm[:], ap_pad[:, 2:2 + W],
                                    op=mybir.AluOpType.max)
            nc.vector.tensor_tensor(hs[:], ap_pad[:, 0:W], ap_pad[:, 1:1 + W],
                                    op=mybir.AluOpType.add)
            nc.vector.tensor_tensor(hs[:], hs[:], ap_pad[:, 2:2 + W],
                                    op=mybir.AluOpType.add)
            vm = small.tile([H, W], F32, name=f"vm{which}{b}", tag=f"vm{which}")
            vs = small.tile([H, W], F32, name=f"vs{which}{b}", tag=f"vs{which}")
            for (t_out, t_in) in ((vm, hm), (vs, hs)):
                up = small.tile([H, W], F32, name=f"up{which}{b}_{t_out.name}", tag=f"up{which}")
                dn = small.tile([H, W], F32, name=f"dn{which}{b}_{t_out.name}", tag=f"dn{which}")
                nc.sync.dma_start(up[0:H - 1, :], t_in[1:H, :])
                nc.scalar.dma_start(up[H - 1:H, :], t_in[0:1, :])
                nc.scalar.dma_start(dn[1:H, :], t_in[0:H - 1, :])
                nc.sync.dma_start(dn[0:1, :], t_in[H - 1:H, :])
                op = mybir.AluOpType.max if t_out is vm else mybir.AluOpType.add
                nc.vector.tensor_tensor(t_out[:], t_in[:], up[:], op=op)
                nc.vector.tensor_tensor(t_out[:], t_out[:], dn[:], op=op)
            alive = small.tile([H, W], F32, name=f"al{which}{b}", tag=f"al{which}")
            nc.vector.tensor_scalar(alive[:], vm[:], 0.1, None,
                                    op0=mybir.AluOpType.is_gt)
            avgok = small.tile([H, W], F32, name=f"ag{which}{b}", tag=f"ag{which}")
            nc.vector.tensor_scalar(avgok[:], vs[:], AVG_LT, None,
                                    op0=mybir.AluOpType.is_lt)
            lif = small.tile([H, W], F32, name=f"lf{which}{b}", tag=f"lf{which}")
            nc.vector.tensor_tensor(lif[:], alive[:], avgok[:],
                                    op=mybir.AluOpType.mult)
            return lif

        pre = living(x_ew, "pre")
        post = living(xnew, "post")
        life = small.tile([H, W], F32, name=f"life{b}", tag="life")
        nc.vector.tensor_tensor(life[:], pre[:], post[:], op=mybir.AluOpType.mult)

        nc.vector.tensor_tensor(xnew[:], xnew[:], bcast(life),
                                op=mybir.AluOpType.mult)
        nc.scalar.dma_start(out_dram[b], xnew[:])


# ----------------------------------------------------------------------------
_PROGRAM_CACHE = {}


def _get_program():
    key = (MM_DTYPE, LRELU_MODE)
    if key in _PROGRAM_CACHE:
        return _PROGRAM_CACHE[key]
    nc = bacc.Bacc("TRN2", target_bir_lowering=False, debug=False,
                   num_devices=N_CORES)
    _xdt = BF16 if MM_DTYPE == "bf16hl" else F32
    xpad_in = nc.dram_tensor("xpad_in", [B_LOC, C, PADT], _xdt, kind="ExternalInput").ap()
    xpadl_in = nc.dram_tensor("xpadl_in", [B_LOC, C, PADT], _xdt, kind="ExternalInput").ap()
    xew_in = nc.dram_tensor("xew_in", [B_LOC, H, CW], F32, kind="ExternalInput").ap()
    m_in = nc.dram_tensor("m_in", [H, B_LOC * W], F32, kind="ExternalInput").ap()
    _wadt = BF16 if MM_DTYPE == "bf16hl" else F32
    _warows = 6 * C if MM_DTYPE == "bf16hl" else 3 * C
    wa_in = nc.dram_tensor("wa_in", [_warows, 3 * HID], _wadt, kind="ExternalInput").ap()
    w2_in = nc.dram_tensor("w2_in", [HID, 32], F32, kind="ExternalInput").ap()
    b1_in = nc.dram_tensor("b1_in", [HID, 1], F32, kind="ExternalInput").ap()
    b2_in = nc.dram_tensor("b2_in", [HID, 1], F32, kind="ExternalInput").ap()
    nb1_in = nc.dram_tensor("nb1_in", [HID, 1], F32, kind="ExternalInput").ap()
    out_dram = nc.dram_tensor("out", [B_LOC, H, CW], F32, kind="ExternalOutput").ap()
    scr_drams = [nc.dram_tensor(f"dxscr{b}", [H, C, W], F32).ap()
                 for b in range(B_LOC)]
    with tile.TileContext(nc) as tc:
        _build_kernel(tc, xpad_in, xpadl_in, xew_in, m_in, wa_in, w2_in, b1_in,
                      b2_in, nb1_in, out_dram, scr_drams)
    nc.compile()
    _PROGRAM_CACHE[key] = nc
    return nc


def _host_weights(filters, W1, b1, W2, b2):
    filters = np.asarray(filters, np.float32)
    W1 = np.asarray(W1, np.float32)
    W2 = np.asarray(W2, np.float32)
    # Weff[o, c, di, dj] = sum_f W1[o, c*NF+f] * filters[f, di, dj]
    w1r = W1.reshape(HID, C, NF)                       # [o, c, f]
    weff = np.einsum("ocf,fij->ocij", w1r, filters)    # [o, c, 3, 3]
    wa = np.zeros((3 * C, 3 * HID), np.float32)
    for d in range(3):          # di = d - 1 (stack replica)
        for idx in range(3):    # dj = idx - 1
            wa[d * C:(d + 1) * C, idx * HID:(idx + 1) * HID] = \
                weff[:, :, d, idx].T
    if MM_DTYPE == "bf16hl":
        import ml_dtypes
        wah = wa.astype(ml_dtypes.bfloat16)
        wal = (wa - wah.astype(np.float32)).astype(ml_dtypes.bfloat16)
        wa = np.concatenate([wah, wal], axis=0)     # [96, 384] bf16
    w2p = np.zeros((HID, 32), np.float32)
    w2p[:, :C] = np.asarray(W2, np.float32).T
    b1v = np.asarray(b1, np.float32).reshape(HID, 1)
    b2v = np.zeros((HID, 1), np.float32)
    for j in range(4):
        b2v[32 * j:32 * j + C, 0] = np.asarray(b2, np.float32)
    return wa, w2p, b1v, b2v


def kernel(x, rand_mask, filters, W1, b1, W2, b2, _want_trace=False):
    x = np.asarray(x, np.float32)
    # host-padded image: rows -2..129 (wrap), cols -1..128 (wrap)
    xpad = np.pad(x, ((0, 0), (0, 0), (2, 2), (1, 1)), mode="wrap")
    xpad = np.ascontiguousarray(xpad.reshape(B, C, PADT))
    if MM_DTYPE == "bf16hl":
        import ml_dtypes
        xpad_h = xpad.astype(ml_dtypes.bfloat16)
        xpad_l = (xpad - xpad_h.astype(np.float32)).astype(ml_dtypes.bfloat16)
        xpad, xpad_lo = xpad_h, xpad_l
    else:
        xpad_lo = xpad
    xew = np.ascontiguousarray(
        x.transpose(0, 2, 1, 3).reshape(B, H, CW))
    m = (np.asarray(rand_mask, np.float32) <= np.float32(FIRE_RATE)).astype(np.float32)
    m = m.reshape(B, H, W).transpose(1, 0, 2)   # [H, B, W]
    wa, w2p, b1v, b2v = _host_weights(filters, W1, b1, W2, b2)

    nc = _get_program()
    in_maps = []
    for core in range(N_CORES):
        sl = slice(core * B_LOC, (core + 1) * B_LOC)
        in_maps.append({
            "xpad_in": xpad[sl], "xpadl_in": xpad_lo[sl], "xew_in": xew[sl],
            "m_in": np.ascontiguousarray(m[:, sl, :]).reshape(H, B_LOC * W),
            "wa_in": wa, "w2_in": w2p, "b1_in": b1v, "b2_in": b2v,
            "nb1_in": -b1v,
        })
    res = run_bass_kernel_spmd(nc, in_maps, list(range(N_CORES)),
                               trace=_want_trace)
    out = np.concatenate([res.results[i]["out"] for i in range(N_CORES)], axis=0)
    out = np.ascontiguousarray(
        out.reshape(B, H, C, W).transpose(0, 2, 1, 3))
    if _want_trace:
        return out, res
    return out


# revision 13
# speedup vs baseline: 1.2396x; 1.2396x over previous
"""Trainium2 Bass kernel for one neural-CA (NCA) update step.

Model (per batch element, all f32):
  pre_life  = living_mask(x)                        # 3x3 circular max/avg pools on alpha=x[:,3]
  y         = depthwise 3x3 circular conv of x with 4 filters  -> [C*4, H, W]
  h         = leaky_relu(W1 @ y + b1, 0.01)         # per-pixel MLP, HID=128
  dx        = W2 @ h + b2
  xnew      = x + dx * (rand_mask <= 0.5)
  post_life = living_mask(xnew)
  out       = xnew * (pre_life & post_life)

Strategy (8 NeuronCores, pure data parallel over batch 32 -> 4 per core):
  * Fold conv+W1 into effective weights Weff[o, c, di, dj] (host precompute).
  * Keep a 3-replica row-shifted padded stack xr[48 = 3*16ch, 66*130] in SBUF
    (half-batch granularity); the 3x3 conv+MLP1 becomes 3 accumulating K=48
    matmuls per 512-pixel chunk whose rhs are column-shifted views of xr.
  * Lrelu+bias on ScalarE straight out of PSUM.
  * MLP2 (K=128, M=16->32 zero-padded) col-tiled via tile_position: 4 chunks
    run concurrently in one PSUM bank; one VectorE op evacuates + adds b2.
  * dx bounces through a DRAM scratch into an H-major layout [H, C*W]; the
    elementwise tail + life-mask pools run there with 128-partition tiles and
    free-dim-broadcast per-pixel masks.
  * Matmul dtype switchable f32 <-> f32r (bitcast) for PE speed.
"""

import os
import sys

os.environ.setdefault("JAX_PLATFORMS", "cpu")
for _p in ("/opt/trn_rl_repo", "/root/.axon_site/_ro/trn_rl_repo"):
    if os.path.isdir(_p) and _p not in sys.path:
        sys.path.insert(0, _p)

from contextlib import ExitStack

import numpy as np

import concourse.bass as bass
import concourse.tile as tile
from concourse import bacc, mybir
from concourse._compat import with_exitstack
from concourse.bass_utils import run_bass_kernel_spmd

# ----------------------------------------------------------------------------
# problem constants (hardcoded per spec nn_CAModel_2121713844629)
B, C, H, W = 32, 16, 128, 128
NF, R, K = 4, 1, 3
HID = 128
FIRE_RATE = 0.5
NEG_SLOPE = 0.01
N_CORES = 8
B_LOC = B // N_CORES          # 4 batches per core
ROWS_PER_CHUNK = 4            # 4 image rows = 512 pixels per matmul chunk
CHUNK = ROWS_PER_CHUNK * W    # 512
N_CHUNKS = H // ROWS_PER_CHUNK                 # 32 per batch
GROUPS = N_CHUNKS // 4        # 8 col-tiled MLP2 groups per batch
HALF_ROWS = 64                # image rows per stack half
SROWS = HALF_ROWS + 2         # 66 stack rows (1 halo row each side)
SW = W + 2                    # 130 padded row width
ST = SROWS * SW               # stack free size per partition
CW = C * W                    # 2048, EW free size
PADROWS = H + 4               # host-padded image rows (x rows -2..129)
PADT = PADROWS * SW           # host-padded flat size per channel

MM_DTYPE = os.environ.get("CA_MM_DTYPE", "bf16hl")  # "bf16hl", "f32r" or "f32"
LRELU_MODE = os.environ.get("CA_LRELU", "act")    # "act" (HW Lrelu) or "decomp" (sim-safe)

F32 = mybir.dt.float32
BF16 = mybir.dt.bfloat16
F32MM = mybir.dt.float32r if MM_DTYPE == "f32r" else mybir.dt.float32


def _mm(ap):
    """View an AP in the matmul input dtype."""
    if MM_DTYPE == "f32r":
        return ap.bitcast(mybir.dt.float32r)
    return ap


def _avg_threshold():
    """Smallest f32 s with (np.float32(s)/9 < 0.2) False, as the strict-< bound.

    reference computes (sum/9 < 0.2); we compare (sum < s*) with s* chosen so
    the predicates agree for every f32 sum value.
    """
    lo = np.float32(1.7)
    hi = np.float32(1.9)
    # binary search over f32 values
    for _ in range(80):
        mid = np.float32((lo.astype(np.float64) + hi.astype(np.float64)) / 2)
        if mid / np.float32(9.0) < np.float32(0.2):
            lo = mid
        else:
            hi = mid
    # predicate (sum/9 < 0.2)  <=>  (sum < hi)
    return float(hi)


AVG_LT = _avg_threshold()


# ----------------------------------------------------------------------------
@with_exitstack
def _build_kernel(ctx: ExitStack, tc: "tile.TileContext",
                  xpad_in, xpadl_in, xew_in, m_in, wa_in, w2_in, b1_in, b2_in,
                  nb1_in, out_dram, scr_drams):
    nc = tc.nc
    consts = ctx.enter_context(tc.tile_pool(name="consts", bufs=1))
    stacks = ctx.enter_context(tc.tile_pool(name="stacks", bufs=2))
    hpool = ctx.enter_context(tc.tile_pool(name="hpool", bufs=6))
    ewpool = ctx.enter_context(tc.tile_pool(name="ewpool", bufs=2))
    small = ctx.enter_context(tc.tile_pool(name="small", bufs=1))
    psum_h = ctx.enter_context(tc.tile_pool(name="psum_h", bufs=3, space="PSUM"))
    psum_dx = ctx.enter_context(tc.tile_pool(name="psum_dx", bufs=1, space="PSUM"))

    # --- constants ----------------------------------------------------------
    if MM_DTYPE == "bf16hl":
        # rows 0-47: bf16(WA) ("hi"); rows 48-95: bf16(WA - hi) ("lo")
        wa_t = consts.tile([6 * C, 3 * HID], BF16)
    else:
        wa_t = consts.tile([3 * C, 3 * HID], F32MM)
    w2_t = consts.tile([HID, 32], F32MM)            # W2^T zero-padded to M=32
    b1_t = consts.tile([HID, 1], F32)
    b2_t = consts.tile([HID, 1], F32)               # b2 replicated at 32j+c
    nc.sync.dma_start(wa_t[:], _mm(wa_in[:]))
    nc.sync.dma_start(w2_t[:], _mm(w2_in[:]))
    nc.sync.dma_start(b1_t[:], b1_in[:])
    nc.sync.dma_start(b2_t[:], b2_in[:])
    if LRELU_MODE == "decomp":
        nb1_t = consts.tile([HID, 1], F32)
        nc.sync.dma_start(nb1_t[:], nb1_in[:])
    m_all = consts.tile([H, B_LOC * W], F32)
    nc.sync.dma_start(m_all[:], m_in[:])

    ew_state = {}

    def phase_A(b):
        """loads + conv + MLP1 + MLP2 + evac + dumps for batch b"""
        scr = scr_drams[b]
        x_ew = ewpool.tile([H, CW], F32, name=f"x_ew{b}", tag="x_ew")
        nc.sync.dma_start(x_ew[:], xew_in[b])
        ew_state[b] = x_ew

        h_tiles = {}
        for s in range(2):
            # stack row r_loc of replica d = x row (64s + r_loc + d - 2)
            #  -> xpad row (64s + r_loc + d), xpad rows = x rows -2..129
            if MM_DTYPE == "bf16hl":
                # hi-stack twice (partitions 0-47 / 48-95) + lo-stack (0-47)
                xr = stacks.tile([6 * C, ST], BF16, name=f"xr{b}_{s}", tag="xr")
                xrl = stacks.tile([3 * C, ST], BF16, name=f"xrl{b}_{s}", tag="xrl")
                for d in range(3):
                    win = HALF_ROWS * s + d
                    srcap = bass.AP(
                        tensor=xpad_in.tensor,
                        offset=xpad_in.offset + (b * C) * PADT + win * SW,
                        ap=[[PADT, C], [1, ST]])
                    for rep in range(2):
                        dstap = bass.AP(tensor=xr.tensor,
                                        offset=xr.offset + (rep * 3 + d) * C * ST,
                                        ap=[[ST, C], [1, ST]])
                        nc.sync.dma_start(dstap, srcap)
                    srcl = bass.AP(
                        tensor=xpadl_in.tensor,
                        offset=xpadl_in.offset + (b * C) * PADT + win * SW,
                        ap=[[PADT, C], [1, ST]])
                    dstl = bass.AP(tensor=xrl.tensor, offset=xrl.offset + d * C * ST,
                                   ap=[[ST, C], [1, ST]])
                    nc.sync.dma_start(dstl, srcl)
            else:
                xr = stacks.tile([3 * C, ST], F32MM, name=f"xr{b}_{s}", tag="xr")
                xrl = None
                for d in range(3):
                    win = HALF_ROWS * s + d
                    srcap = bass.AP(
                        tensor=xpad_in.tensor,
                        offset=xpad_in.offset + (b * C) * PADT + win * SW,
                        ap=[[PADT, C], [1, ST]])
                    dstap = bass.AP(tensor=xr.tensor, offset=xr.offset + d * C * ST,
                                    ap=[[ST, C], [1, ST]])
                    nc.sync.dma_start(dstap, _mm(srcap))

            for cl in range(N_CHUNKS // 2):          # 16 chunks per half
                cg = s * (N_CHUNKS // 2) + cl
                h_ps = psum_h.tile([HID, CHUNK], F32, name=f"hps{b}_{cg}", tag="h_ps")
                base = (ROWS_PER_CHUNK * cl + 1) * SW + 1
                if MM_DTYPE == "bf16hl":
                    for idx in range(3):             # dj = idx - 1
                        # hi*HI + lo*HI  (K = 96: [Wh; Wl] @ [xh; xh])
                        rhs = bass.AP(tensor=xr.tensor,
                                      offset=xr.offset + base + (idx - 1),
                                      ap=[[ST, 6 * C], [SW, ROWS_PER_CHUNK], [1, W]])
                        nc.tensor.matmul(h_ps[:],
                                         wa_t[:, idx * HID:(idx + 1) * HID],
                                         rhs,
                                         start=(idx == 0), stop=False)
                        # hi*LO  (K = 48: Wh @ xl)
                        rhsl = bass.AP(tensor=xrl.tensor,
                                       offset=xrl.offset + base + (idx - 1),
                                       ap=[[ST, 3 * C], [SW, ROWS_PER_CHUNK], [1, W]])
                        nc.tensor.matmul(h_ps[:],
                                         wa_t[0:3 * C, idx * HID:(idx + 1) * HID],
                                         rhsl,
                                         start=False, stop=(idx == 2))
                else:
                    for idx in range(3):             # dj = idx - 1
                        rhs = bass.AP(tensor=xr.tensor,
                                      offset=xr.offset + base + (idx - 1),
                                      ap=[[ST, 3 * C], [SW, ROWS_PER_CHUNK], [1, W]])
                        nc.tensor.matmul(h_ps[:],
                                         wa_t[:, idx * HID:(idx + 1) * HID],
                                         rhs,
                                         start=(idx == 0), stop=(idx == 2))
                h_sb = hpool.tile([HID, CHUNK], F32MM, name=f"h{b}_{cg}", tag="h_sb")
                if LRELU_MODE == "act":
                    nc.scalar.activation(h_sb[:], h_ps[:],
                                         mybir.ActivationFunctionType.Lrelu,
                                         bias=b1_t[:], scale=1.0, alpha=NEG_SLOPE)
                else:
                    # lrelu(v) = relu(v) - slope * relu(-v), v = h + b1
                    rpos = hpool.tile([HID, CHUNK], F32, name=f"rp{b}_{cg}", tag="rpos", bufs=2)
                    rneg = hpool.tile([HID, CHUNK], F32, name=f"rn{b}_{cg}", tag="rneg", bufs=2)
                    nc.scalar.activation(rpos[:], h_ps[:],
                                         mybir.ActivationFunctionType.Relu,
                                         bias=b1_t[:], scale=1.0)
                    nc.scalar.activation(rneg[:], h_ps[:],
                                         mybir.ActivationFunctionType.Relu,
                                         bias=nb1_t[:], scale=-1.0)
                    nc.vector.tensor_scalar(rneg[:], rneg[:], -NEG_SLOPE, None,
                                            op0=mybir.AluOpType.mult)
                    nc.vector.tensor_tensor(h_sb[:], rpos[:], rneg[:],
                                            op=mybir.AluOpType.add)
                h_tiles[cg] = h_sb

                if cg % 4 == 3:                      # one MLP2 group ready
                    t = cg // 4
                    dx_ps = psum_dx.tile([C, 4 * CHUNK], F32, name=f"dxps{b}_{t}", tag="dx_ps")
                    for j in range(4):
                        nc.tensor.matmul(dx_ps[:, j * CHUNK:(j + 1) * CHUNK],
                                         w2_t[:, 0:C], h_tiles[t * 4 + j][:],
                                         start=True, stop=True)
                    dxs = hpool.tile([C, 4 * CHUNK], F32, name=f"dxs{b}_{t}", tag="dxs", bufs=2)
                    if t % 2 == 0:
                        nc.vector.tensor_scalar(dxs[:], dx_ps[:], b2_t[0:C, :], None,
                                                op0=mybir.AluOpType.add)
                    else:
                        nc.scalar.activation(dxs[:], dx_ps[:],
                                             mybir.ActivationFunctionType.Identity,
                                             bias=b2_t[0:C, :], scale=1.0)
                    # dump into H-major DRAM scratch [H, C, W] (idle POOL queue)
                    for j in range(4):
                        srcap = bass.AP(tensor=dxs.tensor,
                                        offset=dxs.offset + j * CHUNK,
                                        ap=[[4 * CHUNK, C], [W, ROWS_PER_CHUNK], [1, W]])
                        dstap = bass.AP(tensor=scr.tensor,
                                        offset=scr.offset + (16 * t + 4 * j) * CW,
                                        ap=[[W, C], [CW, ROWS_PER_CHUNK], [1, W]])
                        nc.gpsimd.dma_start(dstap, srcap)

    def phase_B(b):
        """reload + elementwise tail + life masks + store for batch b"""
        scr = scr_drams[b]
        x_ew = ew_state.pop(b)
        dx_ew = ewpool.tile([H, CW], F32, name=f"dx_ew{b}", tag="dx_ew")
        nc.scalar.dma_start(dx_ew[:], scr.rearrange("h c w -> h (c w)"))

        def bcast(t128):
            return bass.AP(tensor=t128.tensor, offset=t128.offset,
                           ap=[[t128.ap[0][0], H], [0, C], [1, W]])

        m_b = bass.AP(tensor=m_all.tensor, offset=m_all.offset + b * W,
                      ap=[[m_all.ap[0][0], H], [0, C], [1, W]])
        nc.vector.tensor_tensor(dx_ew[:], dx_ew[:], m_b, op=mybir.AluOpType.mult)
        xnew = ewpool.tile([H, CW], F32, name=f"xnew{b}", tag="xnew")
        nc.vector.tensor_tensor(xnew[:], x_ew[:], dx_ew[:], op=mybir.AluOpType.add)

        def living(src_ew, which):
            ap_pad = small.tile([H, SW], F32, name=f"ap{which}{b}", tag=f"ap{which}")
            alpha = src_ew[:, 3 * W:4 * W]
            nc.vector.tensor_copy(ap_pad[:, 1:1 + W], alpha)
            nc.vector.tensor_copy(ap_pad[:, 0:1], src_ew[:, 4 * W - 1:4 * W])
            nc.vector.tensor_copy(ap_pad[:, 1 + W:2 + W], src_ew[:, 3 * W:3 * W + 1])
            hm = small.tile([H, W], F32, name=f"hm{which}{b}", tag=f"hm{which}")
            hs = small.tile([H, W], F32, name=f"hs{which}{b}", tag=f"hs{which}")
            nc.vector.tensor_tensor(hm[:], ap_pad[:, 0:W], ap_pad[:, 1:1 + W],
                                    op=mybir.AluOpType.max)
            nc.vector.tensor_tensor(hm[:], hm[:], ap_pad[:, 2:2 + W],
                                    op=mybir.AluOpType.max)
            nc.vector.tensor_tensor(hs[:], ap_pad[:, 0:W], ap_pad[:, 1:1 + W],
                                    op=mybir.AluOpType.add)
            nc.vector.tensor_tensor(hs[:], hs[:], ap_pad[:, 2:2 + W],
                                    op=mybir.AluOpType.add)
            vm = small.tile([H, W], F32, name=f"vm{which}{b}", tag=f"vm{which}")
            vs = small.tile([H, W], F32, name=f"vs{which}{b}", tag=f"vs{which}")
            for (t_out, t_in) in ((vm, hm), (vs, hs)):
                up = small.tile([H, W], F32, name=f"up{which}{b}_{t_out.name}", tag=f"up{which}")
                dn = small.tile([H, W], F32, name=f"dn{which}{b}_{t_out.name}", tag=f"dn{which}")
                nc.sync.dma_start(up[0:H - 1, :], t_in[1:H, :])
                nc.scalar.dma_start(up[H - 1:H, :], t_in[0:1, :])
                nc.scalar.dma_start(dn[1:H, :], t_in[0:H - 1, :])
                nc.sync.dma_start(dn[0:1, :], t_in[H - 1:H, :])
                op = mybir.AluOpType.max if t_out is vm else mybir.AluOpType.add
                nc.vector.tensor_tensor(t_out[:], t_in[:], up[:], op=op)
                nc.vector.tensor_tensor(t_out[:], t_out[:], dn[:], op=op)
            alive = small.tile([H, W], F32, name=f"al{which}{b}", tag=f"al{which}")
            nc.vector.tensor_scalar(alive[:], vm[:], 0.1, None,
                                    op0=mybir.AluOpType.is_gt)
            avgok = small.tile([H, W], F32, name=f"ag{which}{b}", tag=f"ag{which}")
            nc.vector.tensor_scalar(avgok[:], vs[:], AVG_LT, None,
                                    op0=mybir.AluOpType.is_lt)
            lif = small.tile([H, W], F32, name=f"lf{which}{b}", tag=f"lf{which}")
            nc.vector.tensor_tensor(lif[:], alive[:], avgok[:],
                                    op=mybir.AluOpType.mult)
            return lif

        pre = living(x_ew, "pre")
        post = living(xnew, "post")
        life = small.tile([H, W], F32, name=f"life{b}", tag="life")
        nc.vector.tensor_tensor(life[:], pre[:], post[:], op=mybir.AluOpType.mult)

        nc.vector.tensor_tensor(xnew[:], xnew[:], bcast(life),
                                op=mybir.AluOpType.mult)
        nc.scalar.dma_start(out_dram[b], xnew[:])

    # software pipeline: A(0) A(1) B(0) A(2) B(1) A(3) B(2) B(3)
    phase_A(0)
    for b in range(1, B_LOC):
        phase_A(b)
        phase_B(b - 1)
    phase_B(B_LOC - 1)


# ----------------------------------------------------------------------------
_PROGRAM_CACHE = {}


def _get_program():
    key = (MM_DTYPE, LRELU_MODE)
    if key in _PROGRAM_CACHE:
        return _PROGRAM_CACHE[key]
    nc = bacc.Bacc("TRN2", target_bir_lowering=False, debug=False,
                   num_devices=N_CORES)
    _xdt = BF16 if MM_DTYPE == "bf16hl" else F32
    xpad_in = nc.dram_tensor("xpad_in", [B_LOC, C, PADT], _xdt, kind="ExternalInput").ap()
    xpadl_in = nc.dram_tensor("xpadl_in", [B_LOC, C, PADT], _xdt, kind="ExternalInput").ap()
    xew_in = nc.dram_tensor("xew_in", [B_LOC, H, CW], F32, kind="ExternalInput").ap()
    m_in = nc.dram_tensor("m_in", [H, B_LOC * W], F32, kind="ExternalInput").ap()
    _wadt = BF16 if MM_DTYPE == "bf16hl" else F32
    _warows = 6 * C if MM_DTYPE == "bf16hl" else 3 * C
    wa_in = nc.dram_tensor("wa_in", [_warows, 3 * HID], _wadt, kind="ExternalInput").ap()
    w2_in = nc.dram_tensor("w2_in", [HID, 32], F32, kind="ExternalInput").ap()
    b1_in = nc.dram_tensor("b1_in", [HID, 1], F32, kind="ExternalInput").ap()
    b2_in = nc.dram_tensor("b2_in", [HID, 1], F32, kind="ExternalInput").ap()
    nb1_in = nc.dram_tensor("nb1_in", [HID, 1], F32, kind="ExternalInput").ap()
    out_dram = nc.dram_tensor("out", [B_LOC, H, CW], F32, kind="ExternalOutput").ap()
    scr_drams = [nc.dram_tensor(f"dxscr{b}", [H, C, W], F32).ap()
                 for b in range(B_LOC)]
    with tile.TileContext(nc) as tc:
        _build_kernel(tc, xpad_in, xpadl_in, xew_in, m_in, wa_in, w2_in, b1_in,
                      b2_in, nb1_in, out_dram, scr_drams)
    nc.compile()
    _PROGRAM_CACHE[key] = nc
    return nc


def _host_weights(filters, W1, b1, W2, b2):
    filters = np.asarray(filters, np.float32)
    W1 = np.asarray(W1, np.float32)
    W2 = np.asarray(W2, np.float32)
    # Weff[o, c, di, dj] = sum_f W1[o, c*NF+f] * filters[f, di, dj]
    w1r = W1.reshape(HID, C, NF)                       # [o, c, f]
    weff = np.einsum("ocf,fij->ocij", w1r, filters)    # [o, c, 3, 3]
    wa = np.zeros((3 * C, 3 * HID), np.float32)
    for d in range(3):          # di = d - 1 (stack replica)
        for idx in range(3):    # dj = idx - 1
            wa[d * C:(d + 1) * C, idx * HID:(idx + 1) * HID] = \
                weff[:, :, d, idx].T
    if MM_DTYPE == "bf16hl":
        import ml_dtypes
        wah = wa.astype(ml_dtypes.bfloat16)
        wal = (wa - wah.astype(np.float32)).astype(ml_dtypes.bfloat16)
        wa = np.concatenate([wah, wal], axis=0)     # [96, 384] bf16
    w2p = np.zeros((HID, 32), np.float32)
    w2p[:, :C] = np.asarray(W2, np.float32).T
    b1v = np.asarray(b1, np.float32).reshape(HID, 1)
    b2v = np.zeros((HID, 1), np.float32)
    for j in range(4):
        b2v[32 * j:32 * j + C, 0] = np.asarray(b2, np.float32)
    return wa, w2p, b1v, b2v


def kernel(x, rand_mask, filters, W1, b1, W2, b2, _want_trace=False):
    x = np.asarray(x, np.float32)
    # host-padded image: rows -2..129 (wrap), cols -1..128 (wrap)
    xpad = np.pad(x, ((0, 0), (0, 0), (2, 2), (1, 1)), mode="wrap")
    xpad = np.ascontiguousarray(xpad.reshape(B, C, PADT))
    if MM_DTYPE == "bf16hl":
        import ml_dtypes
        xpad_h = xpad.astype(ml_dtypes.bfloat16)
        xpad_l = (xpad - xpad_h.astype(np.float32)).astype(ml_dtypes.bfloat16)
        xpad, xpad_lo = xpad_h, xpad_l
    else:
        xpad_lo = xpad
    xew = np.ascontiguousarray(
        x.transpose(0, 2, 1, 3).reshape(B, H, CW))
    m = (np.asarray(rand_mask, np.float32) <= np.float32(FIRE_RATE)).astype(np.float32)
    m = m.reshape(B, H, W).transpose(1, 0, 2)   # [H, B, W]
    wa, w2p, b1v, b2v = _host_weights(filters, W1, b1, W2, b2)

    nc = _get_program()
    in_maps = []
    for core in range(N_CORES):
        sl = slice(core * B_LOC, (core + 1) * B_LOC)
        in_maps.append({
            "xpad_in": xpad[sl], "xpadl_in": xpad_lo[sl], "xew_in": xew[sl],
            "m_in": np.ascontiguousarray(m[:, sl, :]).reshape(H, B_LOC * W),
            "wa_in": wa, "w2_in": w2p, "b1_in": b1v, "b2_in": b2v,
            "nb1_in": -b1v,
        })
    res = run_bass_kernel_spmd(nc, in_maps, list(range(N_CORES)),
                               trace=_want_trace)
    out = np.concatenate([res.results[i]["out"] for i in range(N_CORES)], axis=0)
    out = np.ascontiguousarray(
        out.reshape(B, H, C, W).transpose(0, 2, 1, 3))
    if _want_trace:
        return out, res
    return out


# revision 14
# speedup vs baseline: 1.3094x; 1.0564x over previous
"""Trainium2 Bass kernel for one neural-CA (NCA) update step.

Model (per batch element, all f32):
  pre_life  = living_mask(x)                        # 3x3 circular max/avg pools on alpha=x[:,3]
  y         = depthwise 3x3 circular conv of x with 4 filters  -> [C*4, H, W]
  h         = leaky_relu(W1 @ y + b1, 0.01)         # per-pixel MLP, HID=128
  dx        = W2 @ h + b2
  xnew      = x + dx * (rand_mask <= 0.5)
  post_life = living_mask(xnew)
  out       = xnew * (pre_life & post_life)

Strategy (8 NeuronCores, pure data parallel over batch 32 -> 4 per core):
  * Fold conv+W1 into effective weights Weff[o, c, di, dj] (host precompute).
  * Keep a 3-replica row-shifted padded stack xr[48 = 3*16ch, 66*130] in SBUF
    (half-batch granularity); the 3x3 conv+MLP1 becomes 3 accumulating K=48
    matmuls per 512-pixel chunk whose rhs are column-shifted views of xr.
  * Lrelu+bias on ScalarE straight out of PSUM.
  * MLP2 (K=128, M=16->32 zero-padded) col-tiled via tile_position: 4 chunks
    run concurrently in one PSUM bank; one VectorE op evacuates + adds b2.
  * dx bounces through a DRAM scratch into an H-major layout [H, C*W]; the
    elementwise tail + life-mask pools run there with 128-partition tiles and
    free-dim-broadcast per-pixel masks.
  * Matmul dtype switchable f32 <-> f32r (bitcast) for PE speed.
"""

import os
import sys

os.environ.setdefault("JAX_PLATFORMS", "cpu")
for _p in ("/opt/trn_rl_repo", "/root/.axon_site/_ro/trn_rl_repo"):
    if os.path.isdir(_p) and _p not in sys.path:
        sys.path.insert(0, _p)

from contextlib import ExitStack

import numpy as np

import concourse.bass as bass
import concourse.tile as tile
from concourse import bacc, mybir
from concourse._compat import with_exitstack
from concourse.bass_utils import run_bass_kernel_spmd

# ----------------------------------------------------------------------------
# problem constants (hardcoded per spec nn_CAModel_2121713844629)
B, C, H, W = 32, 16, 128, 128
NF, R, K = 4, 1, 3
HID = 128
FIRE_RATE = 0.5
NEG_SLOPE = 0.01
N_CORES = 8
B_LOC = B // N_CORES          # 4 batches per core
ROWS_PER_CHUNK = 4            # 4 image rows = 512 pixels per matmul chunk
CHUNK = ROWS_PER_CHUNK * W    # 512
N_CHUNKS = H // ROWS_PER_CHUNK                 # 32 per batch
GROUPS = N_CHUNKS // 4        # 8 col-tiled MLP2 groups per batch
HALF_ROWS = 64                # image rows per stack half
SROWS = HALF_ROWS + 2         # 66 stack rows (1 halo row each side)
SW = W + 2                    # 130 padded row width
ST = SROWS * SW               # stack free size per partition
CW = C * W                    # 2048, EW free size
PADROWS = H + 4               # host-padded image rows (x rows -2..129)
PADT = PADROWS * SW           # host-padded flat size per channel

MM_DTYPE = os.environ.get("CA_MM_DTYPE", "bf16hl")  # "bf16hl", "f32r" or "f32"
LRELU_MODE = os.environ.get("CA_LRELU", "act")    # "act" (HW Lrelu) or "decomp" (sim-safe)

F32 = mybir.dt.float32
BF16 = mybir.dt.bfloat16
F32MM = mybir.dt.float32r if MM_DTYPE == "f32r" else mybir.dt.float32


def _mm(ap):
    """View an AP in the matmul input dtype."""
    if MM_DTYPE == "f32r":
        return ap.bitcast(mybir.dt.float32r)
    return ap


def _avg_threshold():
    """Smallest f32 s with (np.float32(s)/9 < 0.2) False, as the strict-< bound.

    reference computes (sum/9 < 0.2); we compare (sum < s*) with s* chosen so
    the predicates agree for every f32 sum value.
    """
    lo = np.float32(1.7)
    hi = np.float32(1.9)
    # binary search over f32 values
    for _ in range(80):
        mid = np.float32((lo.astype(np.float64) + hi.astype(np.float64)) / 2)
        if mid / np.float32(9.0) < np.float32(0.2):
            lo = mid
        else:
            hi = mid
    # predicate (sum/9 < 0.2)  <=>  (sum < hi)
    return float(hi)


AVG_LT = _avg_threshold()


# ----------------------------------------------------------------------------
@with_exitstack
def _build_kernel(ctx: ExitStack, tc: "tile.TileContext",
                  xpad_in, xpadl_in, xew_in, m_in, wa_in, w2_in, b1_in, b2_in,
                  nb1_in, out_dram, scr_drams):
    nc = tc.nc
    consts = ctx.enter_context(tc.tile_pool(name="consts", bufs=1))
    stacks = ctx.enter_context(tc.tile_pool(name="stacks", bufs=2))
    hpool = ctx.enter_context(tc.tile_pool(name="hpool", bufs=6))
    ewpool = ctx.enter_context(tc.tile_pool(name="ewpool", bufs=2))
    small = ctx.enter_context(tc.tile_pool(name="small", bufs=1))
    psum_h = ctx.enter_context(tc.tile_pool(name="psum_h", bufs=4, space="PSUM"))
    psum_dx = ctx.enter_context(tc.tile_pool(name="psum_dx", bufs=1, space="PSUM"))

    # --- constants ----------------------------------------------------------
    if MM_DTYPE == "bf16hl":
        # rows 0-47: bf16(WA) ("hi"); rows 48-95: bf16(WA - hi) ("lo")
        wa_t = consts.tile([6 * C, 3 * HID], BF16)
    else:
        wa_t = consts.tile([3 * C, 3 * HID], F32MM)
    w2_t = consts.tile([HID, 32], F32MM)            # W2^T zero-padded to M=32
    b1_t = consts.tile([HID, 1], F32)
    b2_t = consts.tile([HID, 1], F32)               # b2 replicated at 32j+c
    nc.sync.dma_start(wa_t[:], _mm(wa_in[:]))
    nc.sync.dma_start(w2_t[:], _mm(w2_in[:]))
    nc.sync.dma_start(b1_t[:], b1_in[:])
    nc.sync.dma_start(b2_t[:], b2_in[:])
    if LRELU_MODE == "decomp":
        nb1_t = consts.tile([HID, 1], F32)
        nc.sync.dma_start(nb1_t[:], nb1_in[:])
    m_all = consts.tile([H, B_LOC * W], F32)
    nc.sync.dma_start(m_all[:], m_in[:])

    ew_state = {}

    def phase_A(b):
        """loads + conv + MLP1 + MLP2 + evac + dumps for batch b"""
        scr = scr_drams[b]
        x_ew = None

        h_tiles = {}
        for s in range(2):
            # stack row r_loc of replica d = x row (64s + r_loc + d - 2)
            #  -> xpad row (64s + r_loc + d), xpad rows = x rows -2..129
            if MM_DTYPE == "bf16hl":
                # hi-stack twice (partitions 0-47 / 48-95) + lo-stack (0-47)
                xr = stacks.tile([6 * C, ST], BF16, name=f"xr{b}_{s}", tag="xr")
                xrl = stacks.tile([3 * C, ST], BF16, name=f"xrl{b}_{s}", tag="xrl")
                for d in range(3):
                    win = HALF_ROWS * s + d
                    srcap = bass.AP(
                        tensor=xpad_in.tensor,
                        offset=xpad_in.offset + (b * C) * PADT + win * SW,
                        ap=[[PADT, C], [1, ST]])
                    for rep in range(2):
                        dstap = bass.AP(tensor=xr.tensor,
                                        offset=xr.offset + (rep * 3 + d) * C * ST,
                                        ap=[[ST, C], [1, ST]])
                        nc.sync.dma_start(dstap, srcap)
                    srcl = bass.AP(
                        tensor=xpadl_in.tensor,
                        offset=xpadl_in.offset + (b * C) * PADT + win * SW,
                        ap=[[PADT, C], [1, ST]])
                    dstl = bass.AP(tensor=xrl.tensor, offset=xrl.offset + d * C * ST,
                                   ap=[[ST, C], [1, ST]])
                    nc.sync.dma_start(dstl, srcl)
            else:
                xr = stacks.tile([3 * C, ST], F32MM, name=f"xr{b}_{s}", tag="xr")
                xrl = None
                for d in range(3):
                    win = HALF_ROWS * s + d
                    srcap = bass.AP(
                        tensor=xpad_in.tensor,
                        offset=xpad_in.offset + (b * C) * PADT + win * SW,
                        ap=[[PADT, C], [1, ST]])
                    dstap = bass.AP(tensor=xr.tensor, offset=xr.offset + d * C * ST,
                                    ap=[[ST, C], [1, ST]])
                    nc.sync.dma_start(dstap, _mm(srcap))

            if s == 1:
                # issued after the stack loads so it can't block them on SP
                x_ew = ewpool.tile([H, CW], F32, name=f"x_ew{b}", tag="x_ew", bufs=3)
                nc.sync.dma_start(x_ew[:], xew_in[b])
                ew_state[b] = x_ew

            for cl in range(N_CHUNKS // 2):          # 16 chunks per half
                cg = s * (N_CHUNKS // 2) + cl
                h_ps = psum_h.tile([HID, CHUNK], F32, name=f"hps{b}_{cg}", tag="h_ps")
                base = (ROWS_PER_CHUNK * cl + 1) * SW + 1
                if MM_DTYPE == "bf16hl":
                    for idx in range(3):             # dj = idx - 1
                        # hi*HI + lo*HI  (K = 96: [Wh; Wl] @ [xh; xh])
                        rhs = bass.AP(tensor=xr.tensor,
                                      offset=xr.offset + base + (idx - 1),
                                      ap=[[ST, 6 * C], [SW, ROWS_PER_CHUNK], [1, W]])
                        nc.tensor.matmul(h_ps[:],
                                         wa_t[:, idx * HID:(idx + 1) * HID],
                                         rhs,
                                         start=(idx == 0), stop=False)
                        # hi*LO  (K = 48: Wh @ xl)
                        rhsl = bass.AP(tensor=xrl.tensor,
                                       offset=xrl.offset + base + (idx - 1),
                                       ap=[[ST, 3 * C], [SW, ROWS_PER_CHUNK], [1, W]])
                        nc.tensor.matmul(h_ps[:],
                                         wa_t[0:3 * C, idx * HID:(idx + 1) * HID],
                                         rhsl,
                                         start=False, stop=(idx == 2))
                else:
                    for idx in range(3):             # dj = idx - 1
                        rhs = bass.AP(tensor=xr.tensor,
                                      offset=xr.offset + base + (idx - 1),
                                      ap=[[ST, 3 * C], [SW, ROWS_PER_CHUNK], [1, W]])
                        nc.tensor.matmul(h_ps[:],
                                         wa_t[:, idx * HID:(idx + 1) * HID],
                                         rhs,
                                         start=(idx == 0), stop=(idx == 2))
                h_sb = hpool.tile([HID, CHUNK], F32MM, name=f"h{b}_{cg}", tag="h_sb", bufs=8)
                if LRELU_MODE == "act":
                    nc.scalar.activation(h_sb[:], h_ps[:],
                                         mybir.ActivationFunctionType.Lrelu,
                                         bias=b1_t[:], scale=1.0, alpha=NEG_SLOPE)
                else:
                    # lrelu(v) = relu(v) - slope * relu(-v), v = h + b1
                    rpos = hpool.tile([HID, CHUNK], F32, name=f"rp{b}_{cg}", tag="rpos", bufs=2)
                    rneg = hpool.tile([HID, CHUNK], F32, name=f"rn{b}_{cg}", tag="rneg", bufs=2)
                    nc.scalar.activation(rpos[:], h_ps[:],
                                         mybir.ActivationFunctionType.Relu,
                                         bias=b1_t[:], scale=1.0)
                    nc.scalar.activation(rneg[:], h_ps[:],
                                         mybir.ActivationFunctionType.Relu,
                                         bias=nb1_t[:], scale=-1.0)
                    nc.vector.tensor_scalar(rneg[:], rneg[:], -NEG_SLOPE, None,
                                            op0=mybir.AluOpType.mult)
                    nc.vector.tensor_tensor(h_sb[:], rpos[:], rneg[:],
                                            op=mybir.AluOpType.add)
                h_tiles[cg] = h_sb

                if cg % 4 == 3:                      # one MLP2 group ready
                    t = cg // 4
                    dx_ps = psum_dx.tile([C, 4 * CHUNK], F32, name=f"dxps{b}_{t}", tag="dx_ps")
                    for j in range(4):
                        nc.tensor.matmul(dx_ps[:, j * CHUNK:(j + 1) * CHUNK],
                                         w2_t[:, 0:C], h_tiles[t * 4 + j][:],
                                         start=True, stop=True)
                    dxs = hpool.tile([C, 4 * CHUNK], F32, name=f"dxs{b}_{t}", tag="dxs", bufs=2)
                    if t % 2 == 0:
                        nc.vector.tensor_scalar(dxs[:], dx_ps[:], b2_t[0:C, :], None,
                                                op0=mybir.AluOpType.add)
                    else:
                        nc.scalar.activation(dxs[:], dx_ps[:],
                                             mybir.ActivationFunctionType.Identity,
                                             bias=b2_t[0:C, :], scale=1.0)
                    # dump into H-major DRAM scratch [H, C, W] (idle POOL queue)
                    for j in range(4):
                        srcap = bass.AP(tensor=dxs.tensor,
                                        offset=dxs.offset + j * CHUNK,
                                        ap=[[4 * CHUNK, C], [W, ROWS_PER_CHUNK], [1, W]])
                        dstap = bass.AP(tensor=scr.tensor,
                                        offset=scr.offset + (16 * t + 4 * j) * CW,
                                        ap=[[W, C], [CW, ROWS_PER_CHUNK], [1, W]])
                        nc.gpsimd.dma_start(dstap, srcap)

    def phase_B(b):
        """reload + elementwise tail + life masks + store for batch b"""
        scr = scr_drams[b]
        x_ew = ew_state.pop(b)
        dx_ew = ewpool.tile([H, CW], F32, name=f"dx_ew{b}", tag="dx_ew")
        nc.scalar.dma_start(dx_ew[:], scr.rearrange("h c w -> h (c w)"))

        def bcast(t128):
            return bass.AP(tensor=t128.tensor, offset=t128.offset,
                           ap=[[t128.ap[0][0], H], [0, C], [1, W]])

        m_b = bass.AP(tensor=m_all.tensor, offset=m_all.offset + b * W,
                      ap=[[m_all.ap[0][0], H], [0, C], [1, W]])
        nc.vector.tensor_tensor(dx_ew[:], dx_ew[:], m_b, op=mybir.AluOpType.mult)
        xnew = ewpool.tile([H, CW], F32, name=f"xnew{b}", tag="xnew")
        nc.vector.tensor_tensor(xnew[:], x_ew[:], dx_ew[:], op=mybir.AluOpType.add)

        def living(src_ew, which):
            ap_pad = small.tile([H, SW], F32, name=f"ap{which}{b}", tag=f"ap{which}")
            alpha = src_ew[:, 3 * W:4 * W]
            nc.vector.tensor_copy(ap_pad[:, 1:1 + W], alpha)
            nc.vector.tensor_copy(ap_pad[:, 0:1], src_ew[:, 4 * W - 1:4 * W])
            nc.vector.tensor_copy(ap_pad[:, 1 + W:2 + W], src_ew[:, 3 * W:3 * W + 1])
            hm = small.tile([H, W], F32, name=f"hm{which}{b}", tag=f"hm{which}")
            hs = small.tile([H, W], F32, name=f"hs{which}{b}", tag=f"hs{which}")
            nc.vector.tensor_tensor(hm[:], ap_pad[:, 0:W], ap_pad[:, 1:1 + W],
                                    op=mybir.AluOpType.max)
            nc.vector.tensor_tensor(hm[:], hm[:], ap_pad[:, 2:2 + W],
                                    op=mybir.AluOpType.max)
            nc.vector.tensor_tensor(hs[:], ap_pad[:, 0:W], ap_pad[:, 1:1 + W],
                                    op=mybir.AluOpType.add)
            nc.vector.tensor_tensor(hs[:], hs[:], ap_pad[:, 2:2 + W],
                                    op=mybir.AluOpType.add)
            vm = small.tile([H, W], F32, name=f"vm{which}{b}", tag=f"vm{which}")
            vs = small.tile([H, W], F32, name=f"vs{which}{b}", tag=f"vs{which}")
            for (t_out, t_in) in ((vm, hm), (vs, hs)):
                up = small.tile([H, W], F32, name=f"up{which}{b}_{t_out.name}", tag=f"up{which}")
                dn = small.tile([H, W], F32, name=f"dn{which}{b}_{t_out.name}", tag=f"dn{which}")
                nc.sync.dma_start(up[0:H - 1, :], t_in[1:H, :])
                nc.scalar.dma_start(up[H - 1:H, :], t_in[0:1, :])
                nc.scalar.dma_start(dn[1:H, :], t_in[0:H - 1, :])
                nc.sync.dma_start(dn[0:1, :], t_in[H - 1:H, :])
                op = mybir.AluOpType.max if t_out is vm else mybir.AluOpType.add
                nc.vector.tensor_tensor(t_out[:], t_in[:], up[:], op=op)
                nc.vector.tensor_tensor(t_out[:], t_out[:], dn[:], op=op)
            alive = small.tile([H, W], F32, name=f"al{which}{b}", tag=f"al{which}")
            nc.vector.tensor_scalar(alive[:], vm[:], 0.1, None,
                                    op0=mybir.AluOpType.is_gt)
            avgok = small.tile([H, W], F32, name=f"ag{which}{b}", tag=f"ag{which}")
            nc.vector.tensor_scalar(avgok[:], vs[:], AVG_LT, None,
                                    op0=mybir.AluOpType.is_lt)
            lif = small.tile([H, W], F32, name=f"lf{which}{b}", tag=f"lf{which}")
            nc.vector.tensor_tensor(lif[:], alive[:], avgok[:],
                                    op=mybir.AluOpType.mult)
            return lif

        pre = living(x_ew, "pre")
        post = living(xnew, "post")
        life = small.tile([H, W], F32, name=f"life{b}", tag="life")
        nc.vector.tensor_tensor(life[:], pre[:], post[:], op=mybir.AluOpType.mult)

        nc.vector.tensor_tensor(xnew[:], xnew[:], bcast(life),
                                op=mybir.AluOpType.mult)
        nc.scalar.dma_start(out_dram[b], xnew[:])

    # software pipeline: A(0) A(1) B(0) A(2) B(1) A(3) B(2) B(3)
    phase_A(0)
    for b in range(1, B_LOC):
        phase_A(b)
        phase_B(b - 1)
    phase_B(B_LOC - 1)


# ----------------------------------------------------------------------------
_PROGRAM_CACHE = {}


def _get_program():
    key = (MM_DTYPE, LRELU_MODE)
    if key in _PROGRAM_CACHE:
        return _PROGRAM_CACHE[key]
    nc = bacc.Bacc("TRN2", target_bir_lowering=False, debug=False,
                   num_devices=N_CORES)
    _xdt = BF16 if MM_DTYPE == "bf16hl" else F32
    xpad_in = nc.dram_tensor("xpad_in", [B_LOC, C, PADT], _xdt, kind="ExternalInput").ap()
    xpadl_in = nc.dram_tensor("xpadl_in", [B_LOC, C, PADT], _xdt, kind="ExternalInput").ap()
    xew_in = nc.dram_tensor("xew_in", [B_LOC, H, CW], F32, kind="ExternalInput").ap()
    m_in = nc.dram_tensor("m_in", [H, B_LOC * W], F32, kind="ExternalInput").ap()
    _wadt = BF16 if MM_DTYPE == "bf16hl" else F32
    _warows = 6 * C if MM_DTYPE == "bf16hl" else 3 * C
    wa_in = nc.dram_tensor("wa_in", [_warows, 3 * HID], _wadt, kind="ExternalInput").ap()
    w2_in = nc.dram_tensor("w2_in", [HID, 32], F32, kind="ExternalInput").ap()
    b1_in = nc.dram_tensor("b1_in", [HID, 1], F32, kind="ExternalInput").ap()
    b2_in = nc.dram_tensor("b2_in", [HID, 1], F32, kind="ExternalInput").ap()
    nb1_in = nc.dram_tensor("nb1_in", [HID, 1], F32, kind="ExternalInput").ap()
    out_dram = nc.dram_tensor("out", [B_LOC, H, CW], F32, kind="ExternalOutput").ap()
    scr_drams = [nc.dram_tensor(f"dxscr{b}", [H, C, W], F32).ap()
                 for b in range(B_LOC)]
    with tile.TileContext(nc) as tc:
        _build_kernel(tc, xpad_in, xpadl_in, xew_in, m_in, wa_in, w2_in, b1_in,
                      b2_in, nb1_in, out_dram, scr_drams)
    nc.compile()
    _PROGRAM_CACHE[key] = nc
    return nc


def _host_weights(filters, W1, b1, W2, b2):
    filters = np.asarray(filters, np.float32)
    W1 = np.asarray(W1, np.float32)
    W2 = np.asarray(W2, np.float32)
    # Weff[o, c, di, dj] = sum_f W1[o, c*NF+f] * filters[f, di, dj]
    w1r = W1.reshape(HID, C, NF)                       # [o, c, f]
    weff = np.einsum("ocf,fij->ocij", w1r, filters)    # [o, c, 3, 3]
    wa = np.zeros((3 * C, 3 * HID), np.float32)
    for d in range(3):          # di = d - 1 (stack replica)
        for idx in range(3):    # dj = idx - 1
            wa[d * C:(d + 1) * C, idx * HID:(idx + 1) * HID] = \
                weff[:, :, d, idx].T
    if MM_DTYPE == "bf16hl":
        import ml_dtypes
        wah = wa.astype(ml_dtypes.bfloat16)
        wal = (wa - wah.astype(np.float32)).astype(ml_dtypes.bfloat16)
        wa = np.concatenate([wah, wal], axis=0)     # [96, 384] bf16
    w2p = np.zeros((HID, 32), np.float32)
    w2p[:, :C] = np.asarray(W2, np.float32).T
    b1v = np.asarray(b1, np.float32).reshape(HID, 1)
    b2v = np.zeros((HID, 1), np.float32)
    for j in range(4):
        b2v[32 * j:32 * j + C, 0] = np.asarray(b2, np.float32)
    return wa, w2p, b1v, b2v


def kernel(x, rand_mask, filters, W1, b1, W2, b2, _want_trace=False):
    x = np.asarray(x, np.float32)
    # host-padded image: rows -2..129 (wrap), cols -1..128 (wrap)
    xpad = np.pad(x, ((0, 0), (0, 0), (2, 2), (1, 1)), mode="wrap")
    xpad = np.ascontiguousarray(xpad.reshape(B, C, PADT))
    if MM_DTYPE == "bf16hl":
        import ml_dtypes
        xpad_h = xpad.astype(ml_dtypes.bfloat16)
        xpad_l = (xpad - xpad_h.astype(np.float32)).astype(ml_dtypes.bfloat16)
        xpad, xpad_lo = xpad_h, xpad_l
    else:
        xpad_lo = xpad
    xew = np.ascontiguousarray(
        x.transpose(0, 2, 1, 3).reshape(B, H, CW))
    m = (np.asarray(rand_mask, np.float32) <= np.float32(FIRE_RATE)).astype(np.float32)
    m = m.reshape(B, H, W).transpose(1, 0, 2)   # [H, B, W]
    wa, w2p, b1v, b2v = _host_weights(filters, W1, b1, W2, b2)

    nc = _get_program()
    in_maps = []
    for core in range(N_CORES):
        sl = slice(core * B_LOC, (core + 1) * B_LOC)
        in_maps.append({
            "xpad_in": xpad[sl], "xpadl_in": xpad_lo[sl], "xew_in": xew[sl],
            "m_in": np.ascontiguousarray(m[:, sl, :]).reshape(H, B_LOC * W),
            "wa_in": wa, "w2_in": w2p, "b1_in": b1v, "b2_in": b2v,
            "nb1_in": -b1v,
        })
    res = run_bass_kernel_spmd(nc, in_maps, list(range(N_CORES)),
                               trace=_want_trace)
    out = np.concatenate([res.results[i]["out"] for i in range(N_CORES)], axis=0)
    out = np.ascontiguousarray(
        out.reshape(B, H, C, W).transpose(0, 2, 1, 3))
    if _want_trace:
        return out, res
    return out
